# revision 79
# baseline (speedup 1.0000x reference)
"""LoRA first-layer MLP kernel for 8 Trainium2 NeuronCores.

Computation:
    W_eff = W0 + 2.0 * (B @ A)            # [4096, 1024]
    h     = relu(x @ W_eff^T + b0)        # [16384, 4096]
    out   = (h @ W2^T + b2).squeeze(-1)   # [16384]

Sharding: data-parallel over batch; each of the 8 cores handles 2048 rows of
x and replicates the weights. No collectives needed.

Per-core device kernel (bf16 operands, f32 PSUM):
  - W2 is folded into the weights on host (W0/B rows and b0 scaled by
    w2[m], m sign-sorted so positive-w2 rows come first).  Then
      out[b] = sum_P relu(z[b,m]) - sum_N relu(-z[b,m]) + b2,
    z = x @ (w2*W_eff)^T + w2*b0, i.e. fc2 collapses into a free-dim
    reduction that rides the RELU pass.
  - Layer 1 computes z tiles [128 batch, m-block] over 5 m-phases
    (512, 512, 1024, 1024, 1024 wide); the two leading 512 phases let
    the first tiles gate on only 1 MB of W.  PE accumulates 8 d-chunks
    per 512-wide half; lhsT = x^T slice [128d, 128b], rhs = W_eff^T
    block [128d, 512m].
  - LoRA rank-16 correction (w2-scaled 2*(B@A)^T) on device: K=16
    matmuls packed into disjoint 32-row PE bands (tile_position row
    tiling, ~4 concurrent), staged to SBUF via Scalar/Vector copies at
    startup and added into resident W as its DMA chunks land.
  - bias: one in-place DVE add per tile from a host-replicated w2*b0.
  - relu + fc2: ScalarE activation(Relu, scale=+-1, accum_out) per
    sign-pure m-segment; segmented 3D-AP reduce combines on DVE,
    overlapped with the last tile; 32x32 DVE block-transpose + one
    de-interleaving DMA writes the output.
"""

import sys

sys.path.insert(0, "/opt/trn_rl_repo")

import ml_dtypes
import numpy as np

import concourse.bacc as bacc
import concourse.bass as bass
import concourse.mybir as mybir
import concourse.tile as tile
from concourse.bass_utils import run_bass_kernel_spmd

F32 = mybir.dt.float32
BF16 = mybir.dt.bfloat16
NP_BF16 = ml_dtypes.bfloat16

N_CORES = 8
B_FULL, D, M, R = 16384, 1024, 4096, 16
SCALING = 2.0
BS = B_FULL // N_CORES  # 2048 rows per core
NSUB = BS // 128  # 16 batch sub-chunks of 128
ND = D // 128  # 8 d-chunks
NMB = M // 512  # 8 m-blocks of 512
NMP = NMB // 2  # 4 m-pairs of 1024

# m-range per compute phase (pairs of 512-blocks)
PHASES = [(0, 1024), (1024, 2048), (2048, 3072), (3072, 4096)]

_CACHE = {}


def _act_plan(c):
    """Sign-pure activation segments per phase.

    Returns (plan, n_p, n_n): plan[ph] = list of (lo, hi, sign, is_p, col)
    with lo/hi in-phase column offsets, col the per-bsub accum column in
    RSP (is_p) or RSN.
    """
    plan = []
    p_cols = 0
    n_cols = 0
    for mlo, mhi in PHASES:
        w = mhi - mlo
        if c >= mhi:
            segs = [(0, w, 1.0, True)]
        elif c <= mlo:
            segs = [(0, w, -1.0, False)]
        else:
            cb = c - mlo
            segs = [(0, cb, 1.0, True), (cb, w, -1.0, False)]
        out = []
        for lo, hi, sign, is_p in segs:
            if is_p:
                out.append((lo, hi, sign, True, p_cols))
                p_cols += 1
            else:
                out.append((lo, hi, sign, False, n_cols))
                n_cols += 1
        plan.append(out)
    return plan, p_cols, n_cols


def _build_nc(c):
    plan, n_p, n_n = _act_plan(c)

    nc = bacc.Bacc(
        "TRN2",
        target_bir_lowering=False,
        debug=False,
        num_devices=N_CORES,
    )
    xt = nc.dram_tensor("xt", [128, NSUB * 1024], BF16, kind="ExternalInput").ap()
    # free layout: [pair, mb&1, dc, 512] — SBUF-identical, mb-major within
    # a pair so a 512-phase's W is one contiguous region.
    w0t = nc.dram_tensor("w0t", [128, NMB * ND * 512], BF16, kind="ExternalInput").ap()
    abp = nc.dram_tensor("abp", [128, D + M], BF16, kind="ExternalInput").ap()
    b0r = nc.dram_tensor("b0r", [128, M], BF16, kind="ExternalInput").ap()
    b2s = nc.dram_tensor("b2s", [128, 1], F32, kind="ExternalInput").ap()
    # padded: rows 16..31 receive the vector-transpose garbage lanes
    out = nc.dram_tensor("out", [2 * NSUB, 128], F32, kind="ExternalOutput").ap()

    RELU = mybir.ActivationFunctionType.Relu
    ADD = mybir.AluOpType.add
    SUB = mybir.AluOpType.subtract
    MULT = mybir.AluOpType.mult
    AXX = mybir.AxisListType.X

    with tile.TileContext(nc) as tc:
        with (
            tc.tile_pool(name="cp", bufs=1) as cp,
            tc.tile_pool(name="hb", bufs=3) as hb,
            tc.tile_pool(name="rb", bufs=2) as rb,
            tc.tile_pool(name="psh", bufs=2, space="PSUM") as psh,
            tc.tile_pool(name="psl", bufs=1, space="PSUM") as psl,
        ):
            AB = cp.tile([128, D + M], BF16, tag="ab")
            BT = AB[:, 128 : 128 + M]

            def a2s(dc):
                if dc == 0:
                    return AB[:, 0:128]
                return AB[:, 128 + M + (dc - 1) * 128 : 128 + M + dc * 128]
            B0R = cp.tile([128, M], BF16, tag="b0r")
            W = cp.tile([128, NMB * ND * 512], BF16, tag="w")
            X = cp.tile([128, NSUB * 1024], BF16, tag="x")
            B2C = cp.tile([128, 1], F32, tag="b2")
            CORR = cp.tile([128, 8192], BF16, tag="corr")
            # +1 spare column: the last tile splits its boundary-crossing
            # activation segment in two, and accum_out overwrites.
            RSP = cp.tile([128, NSUB * max(n_p, 1) + 1], F32, tag="rsp")
            RSN = cp.tile([128, NSUB * max(n_n, 1) + 1], F32, tag="rsn")
            OUT = cp.tile([128, 2 * NSUB], F32, tag="out")
            OUTT = cp.tile([128, 2 * NSUB], F32, tag="outt")

            def _chunked(dma_fn, lo, hi, step=2048):
                while lo < hi:
                    mid = min(lo + step, hi)
                    dma_fn(lo, mid)
                    lo = mid

            def x_dma(blo, bhi):
                _chunked(
                    lambda lo, hi: nc.sync.dma_start(
                        out=X[:, lo:hi], in_=xt[:, lo:hi]
                    ),
                    blo * 1024,
                    bhi * 1024,
                )

            def w_dma(lo, hi):
                _chunked(
                    lambda lo_, hi_: nc.sync.dma_start(
                        out=W[:, lo_:hi_], in_=w0t[:, lo_:hi_]
                    ),
                    lo,
                    hi,
                )

            def w_off(mb, dc):
                return (mb // 2) * 8192 + (mb % 2) * 4096 + dc * 512

            def lora_unit(mp_, dc):
                """LoRA correction blocks (mb0, dc), (mb1, dc) of pair mp_:
                two K=16 matmuls packed into disjoint PE row bands, one
                single-bank PSUM tile per block (4 tiles in flight)."""
                lps = []
                for i, mb in enumerate((2 * mp_, 2 * mp_ + 1)):
                    # alternate row-band sets per dc parity so consecutive
                    # units hit distinct PE row groups and overlap fully
                    r0 = 64 * (dc % 2) + 32 * i
                    lp = psl.tile([128, 512], F32, tag=f"l{2 * (dc % 2) + i}")
                    nc.tensor.matmul(
                        lp[:],
                        a2s(dc)[r0 : r0 + R, :],
                        BT[r0 : r0 + R, mb * 512 : (mb + 1) * 512],
                        start=True,
                        stop=True,
                        tile_position=(r0, 0),
                    )
                    lps.append(lp)
                return lps

            def lora_group(mp_, dc):
                """In-phase lora: unit matmuls + direct adds into W."""
                lpa, lpb = lora_unit(mp_, dc)
                for mb, lp in ((2 * mp_, lpa), (2 * mp_ + 1, lpb)):
                    off = w_off(mb, dc)
                    nc.vector.tensor_add(
                        W[:, off : off + 512], W[:, off : off + 512], lp[:]
                    )

            # --- prologue DMAs, strict need-order per queue: completions
            # are FIFO, so anything issued earlier delays the semaphore of
            # everything after it.  Sync carries only the tile-gating
            # prefix; scalar/gpsimd queues issue the rest in parallel.
            nc.sync.dma_start(out=AB[:, 0:1152], in_=abp[:, 0:1152])
            nc.sync.dma_start(
                out=AB[:, 128 + M : 128 + M + 896],
                in_=abp[:, 128 + M : 128 + M + 896],
            )
            w_dma(0, 8192)  # pair 0
            x_dma(0, 1)
            x_dma(1, 4)
            nc.scalar.dma_start(out=B0R[:, 0:1024], in_=b0r[:, 0:1024])
            nc.gpsimd.dma_start(out=B2C[:], in_=b2s)
            nc.vector.memset(OUT[:, NSUB : 2 * NSUB], 0.0)

            # PE p-state warmup: dummy matmuls on memset data, runnable
            # from boot (no DMA dependency) so the tensor engine is at
            # full clock when the first real work arrives.  Short fillers
            # are also sprinkled through the prologue (see dummy_mm) —
            # idle gaps reset the clock ramp.
            DUM = cp.tile([128, 640], BF16, tag="dum")
            nc.vector.memset(DUM[:], 0.0)
            _dumk = [0]

            def dummy_mm(cols=512, n=1):
                # fillers cycle the (idle-until-tiles) psh pool so they
                # never contend with the lora psl tiles
                for _ in range(n):
                    dps = psh.tile([128, cols], F32, tag="hp")
                    _dumk[0] += 1
                    nc.tensor.matmul(
                        dps[:],
                        DUM[:, 0:128],
                        DUM[:, 128 : 128 + cols],
                        start=True,
                        stop=True,
                    )

            dummy_mm(cols=512, n=10)

            # LoRA for pair 0: stage corrections in SBUF via Scalar/Vector
            # copies (no W dependency), then cheap bf16 SBUF adds per W
            # chunk as its DMA lands — Vector never head-of-line blocks.
            def corr_add(q):
                off = q * 2048
                nc.vector.tensor_add(
                    W[:, off : off + 2048],
                    W[:, off : off + 2048],
                    CORR[:, off : off + 2048],
                )

            for dc in range(ND):
                lpa, lpb = lora_unit(0, dc)
                nc.scalar.copy(CORR[:, dc * 512 : (dc + 1) * 512], lpa[:])
                nc.vector.tensor_copy(
                    CORR[:, 4096 + dc * 512 : 4096 + (dc + 1) * 512], lpb[:]
                )
                if dc == 3:
                    # dc0-3 corrections complete: unblock the first tiles'
                    # early matmuls (chunks 0 and 2) ahead of the rest
                    corr_add(0)
                    corr_add(2)
            corr_add(1)
            corr_add(3)
            # two fillers bridge the q-add wait so the PE clock stays
            # ramped into the first tiles
            dummy_mm(cols=512, n=2)

            def combine(lo, hi, p_extra=0, n_extra=0):
                """out[b] = sum_P - sum_N + b2 for bsubs [lo, hi) via
                segmented (3D-AP) reduces + one fused scalar op."""
                w = hi - lo
                oc = OUT[:, lo:hi]
                rp = rn = None
                if n_p:
                    rp = rb.tile([128, w], F32, tag="rp")
                    src = RSP[:, lo * n_p : hi * n_p + p_extra]
                    if w > 1:
                        src = src.rearrange("p (b c) -> p b c", c=n_p)
                    nc.vector.tensor_reduce(rp[:], src, AXX, ADD)
                if n_n:
                    rn = rb.tile([128, w], F32, tag="rn")
                    src = RSN[:, lo * n_n : hi * n_n + n_extra]
                    if w > 1:
                        src = src.rearrange("p (b c) -> p b c", c=n_n)
                    nc.vector.tensor_reduce(rn[:], src, AXX, ADD)
                if n_p and n_n:
                    nc.vector.scalar_tensor_tensor(
                        oc, rp[:], B2C[:, 0:1], rn[:], ADD, SUB
                    )
                elif n_p:
                    nc.vector.tensor_scalar_add(oc, rp[:], B2C[:, 0:1])
                else:
                    nc.vector.scalar_tensor_tensor(
                        oc, rn[:], -1.0, B2C[:, 0:1], MULT, ADD
                    )

            for ph, (mlo, mhi) in enumerate(PHASES):
                wph = mhi - mlo
                mbs = [mlo // 512 + k for k in range(wph // 512)]
                last_ph = ph == len(PHASES) - 1
                for bsub in range(NSUB):
                    # paced background DMA / next-phase prep, need-ordered
                    if ph == 0:
                        if bsub == 0:
                            x_dma(4, 10)
                        elif bsub == 4:
                            x_dma(10, NSUB)
                        elif bsub == 8:
                            _chunked(
                                lambda lo, hi: nc.sync.dma_start(
                                    out=AB[:, 128 + lo : 128 + hi],
                                    in_=abp[:, 128 + lo : 128 + hi],
                                ),
                                1024,
                                4096,
                            )
                    if ph <= 2:
                        if bsub == 0:
                            w_dma((ph + 1) * 8192, (ph + 2) * 8192)
                            nc.sync.dma_start(
                                out=B0R[:, (ph + 1) * 1024 : (ph + 2) * 1024],
                                in_=b0r[:, (ph + 1) * 1024 : (ph + 2) * 1024],
                            )
                        if bsub in (8, 10, 12, 14):
                            # adjacent dc pair: their 4 matmuls pack into
                            # the four PE row bands in one span
                            k = bsub - 8
                            lora_group(ph + 1, k)
                            lora_group(ph + 1, k + 1)

                    def mm(hp, si, dc, slot=None):
                        mb = mbs[si]
                        slot = si if slot is None else slot
                        lhsT = X[
                            :, bsub * 1024 + dc * 128 : bsub * 1024 + (dc + 1) * 128
                        ]
                        nc.tensor.matmul(
                            hp[:, slot * 512 : slot * 512 + 512],
                            lhsT,
                            W[:, w_off(mb, dc) : w_off(mb, dc) + 512],
                            start=(dc == 0),
                            stop=(dc == ND - 1),
                        )

                    def bias(hp, lo, hi):
                        nc.vector.tensor_add(
                            hp[:, lo:hi],
                            hp[:, lo:hi],
                            B0R[:, mlo + lo : mlo + hi],
                        )

                    def act(hp, lo, hi, sign, is_p, col, spare=False):
                        hs = hb.tile([128, 1024], BF16, tag="hs")
                        rs = RSP if is_p else RSN
                        ncols = n_p if is_p else n_n
                        c0 = NSUB * ncols if spare else bsub * ncols + col
                        nc.scalar.activation(
                            hs[:, 0 : hi - lo],
                            hp[:, lo:hi],
                            RELU,
                            scale=sign,
                            accum_out=rs[:, c0 : c0 + 1],
                        )

                    if last_ph and bsub == NSUB - 1:
                        # last tile: half-split into two separate PSUM
                        # tiles (tile-granular dep tracking would stall
                        # half-B's matmuls behind half-A's reads); the
                        # boundary-crossing activation segment goes to the
                        # spare accum column.
                        hpa = psh.tile([128, 1024], F32, tag="hp")
                        hpb = psl.tile([128, 512], F32, tag="l0")
                        for dc in range(ND):
                            mm(hpa, 0, dc)
                        for dc in range(ND):
                            mm(hpb, 1, dc, slot=0)
                        bias(hpa, 0, 512)
                        for lo, hi, sign, is_p, col in plan[ph]:
                            if hi <= 512:
                                act(hpa, lo, hi, sign, is_p, col)
                            elif lo < 512:
                                act(hpa, lo, 512, sign, is_p, col)
                        combine(0, NSUB - 1)
                        nc.vector.tensor_add(
                            hpb[:, 0:512],
                            hpb[:, 0:512],
                            B0R[:, mlo + 512 : mlo + 1024],
                        )
                        p_extra = n_extra = 0
                        for lo, hi, sign, is_p, col in plan[ph]:
                            if lo >= 512:
                                act(hpb, lo - 512, hi - 512, sign, is_p, col)
                            elif hi > 512:
                                act(hpb, 0, hi - 512, sign, is_p, col, spare=True)
                                if is_p:
                                    p_extra = 1
                                else:
                                    n_extra = 1
                        combine(NSUB - 1, NSUB, p_extra, n_extra)
                    else:
                        hp = psh.tile([128, wph], F32, tag="hp")
                        for dc in range(ND):
                            for si in range(len(mbs)):
                                mm(hp, si, dc)
                        bias(hp, 0, wph)
                        for lo, hi, sign, is_p, col in plan[ph]:
                            act(hp, lo, hi, sign, is_p, col)

            # output: 32x32 block transpose on VectorE, then one DMA whose
            # DRAM AP de-interleaves the blocks; OUTT[32i+c, r] holds
            # y[bsub=c, b=32i+r], so dram row c gets (i r).
            nc.vector.transpose(OUTT[:], OUT[:])
            nc.sync.dma_start(
                out=out.rearrange("c (i r) -> i c r", i=4),
                in_=OUTT[:, 0 : 2 * NSUB],
            )

    nc.compile()
    return nc


def _prep_in_maps(x, W0, b0, A, B, W2, b2, c, perm):
    w2 = W2[0]
    W0p = (W0 * w2[:, None])[perm]
    Bp = (B * w2[:, None])[perm]
    b0p = (b0 * w2)[perm]

    # [128, mp*8192 + s*4096 + dc*512 + j] = W0p[(2mp+s)*512 + j, dc*128 + p]
    w0t = np.ascontiguousarray(
        W0p.reshape(NMP, 2, 512, ND, 128)
        .transpose(4, 0, 1, 3, 2)
        .reshape(128, NMB * ND * 512)
    ).astype(NP_BF16)
    # lora operands replicated into the four 32-row PE tile bands,
    # concatenated [A2 | BT] so one DMA covers the first unit's needs
    abp = np.zeros((128, D + M), dtype=NP_BF16)
    a2full = (SCALING * A).astype(NP_BF16)
    btfull = Bp.T.astype(NP_BF16)
    for i in range(4):
        abp[32 * i : 32 * i + R, 0:128] = a2full[:, 0:128]
        abp[32 * i : 32 * i + R, 128 : 128 + M] = btfull
        abp[32 * i : 32 * i + R, 128 + M :] = a2full[:, 128:]
    b0rep = np.ascontiguousarray(
        np.broadcast_to(b0p.astype(NP_BF16)[None, :], (128, M))
    )
    b2s = np.full((128, 1), b2[0], dtype=np.float32)

    in_maps = []
    for cix in range(N_CORES):
        xs = x[cix * BS : (cix + 1) * BS]  # [2048, 1024]
        # xt[p, bsub*1024 + dc*128 + bb] = xs[bsub*128 + bb, dc*128 + p]
        xt = np.ascontiguousarray(
            xs.reshape(NSUB, 128, ND, 128).transpose(3, 0, 2, 1).reshape(128, NSUB * 1024)
        ).astype(NP_BF16)
        in_maps.append(
            {
                "xt": xt,
                "w0t": w0t,
                "abp": abp,
                "b0r": b0rep,
                "b2s": b2s,
            }
        )
    return in_maps


def kernel(x, W0, b0, A, B, W2, b2, _trace=False, _trace_kwargs=None):
    x = np.asarray(x, dtype=np.float32)
    W0 = np.asarray(W0, dtype=np.float32)
    b0 = np.asarray(b0, dtype=np.float32)
    A = np.asarray(A, dtype=np.float32)
    B = np.asarray(B, dtype=np.float32)
    W2 = np.asarray(W2, dtype=np.float32)
    b2 = np.asarray(b2, dtype=np.float32)

    w2 = W2[0]
    pos = w2 >= 0
    c = int(pos.sum())
    perm = np.concatenate([np.where(pos)[0], np.where(~pos)[0]])

    key = ("nc", c)
    if key not in _CACHE:
        _CACHE[key] = _build_nc(c)
    nc = _CACHE[key]

    in_maps = _prep_in_maps(x, W0, b0, A, B, W2, b2, c, perm)
    res = run_bass_kernel_spmd(
        nc,
        in_maps,
        list(range(N_CORES)),
        trace=_trace,
        **(_trace_kwargs or {}),
    )
    out = np.concatenate([r["out"][:NSUB].reshape(BS) for r in res.results])
    if _trace:
        _CACHE["last_results"] = res
    return out.astype(np.float32)


# revision 80
# speedup vs baseline: 1.0016x; 1.0016x over previous
"""LoRA first-layer MLP kernel for 8 Trainium2 NeuronCores.

Computation:
    W_eff = W0 + 2.0 * (B @ A)            # [4096, 1024]
    h     = relu(x @ W_eff^T + b0)        # [16384, 4096]
    out   = (h @ W2^T + b2).squeeze(-1)   # [16384]

Sharding: data-parallel over batch; each of the 8 cores handles 2048 rows of
x and replicates the weights. No collectives needed.

Per-core device kernel (bf16 operands, f32 PSUM):
  - W2 is folded into the weights on host (W0/B rows and b0 scaled by
    w2[m], m sign-sorted so positive-w2 rows come first).  Then
      out[b] = sum_P relu(z[b,m]) - sum_N relu(-z[b,m]) + b2,
    z = x @ (w2*W_eff)^T + w2*b0, i.e. fc2 collapses into a free-dim
    reduction that rides the RELU pass.
  - Layer 1 computes z tiles [128 batch, m-block] over 5 m-phases
    (512, 512, 1024, 1024, 1024 wide); the two leading 512 phases let
    the first tiles gate on only 1 MB of W.  PE accumulates 8 d-chunks
    per 512-wide half; lhsT = x^T slice [128d, 128b], rhs = W_eff^T
    block [128d, 512m].
  - LoRA rank-16 correction (w2-scaled 2*(B@A)^T) on device: K=16
    matmuls packed into disjoint 32-row PE bands (tile_position row
    tiling, ~4 concurrent), staged to SBUF via Scalar/Vector copies at
    startup and added into resident W as its DMA chunks land.
  - bias: one in-place DVE add per tile from a host-replicated w2*b0.
  - relu + fc2: ScalarE activation(Relu, scale=+-1, accum_out) per
    sign-pure m-segment; segmented 3D-AP reduce combines on DVE,
    overlapped with the last tile; 32x32 DVE block-transpose + one
    de-interleaving DMA writes the output.
"""

import sys

sys.path.insert(0, "/opt/trn_rl_repo")

import ml_dtypes
import numpy as np

import concourse.bacc as bacc
import concourse.bass as bass
import concourse.mybir as mybir
import concourse.tile as tile
from concourse.bass_utils import run_bass_kernel_spmd

F32 = mybir.dt.float32
BF16 = mybir.dt.bfloat16
NP_BF16 = ml_dtypes.bfloat16

N_CORES = 8
B_FULL, D, M, R = 16384, 1024, 4096, 16
SCALING = 2.0
BS = B_FULL // N_CORES  # 2048 rows per core
NSUB = BS // 128  # 16 batch sub-chunks of 128
ND = D // 128  # 8 d-chunks
NMB = M // 512  # 8 m-blocks of 512
NMP = NMB // 2  # 4 m-pairs of 1024

# m-range per compute phase (pairs of 512-blocks)
PHASES = [(0, 1024), (1024, 2048), (2048, 3072), (3072, 4096)]

_CACHE = {}


def _act_plan(c):
    """Sign-pure activation segments per phase.

    Returns (plan, n_p, n_n): plan[ph] = list of (lo, hi, sign, is_p, col)
    with lo/hi in-phase column offsets, col the per-bsub accum column in
    RSP (is_p) or RSN.
    """
    plan = []
    p_cols = 0
    n_cols = 0
    for mlo, mhi in PHASES:
        w = mhi - mlo
        if c >= mhi:
            segs = [(0, w, 1.0, True)]
        elif c <= mlo:
            segs = [(0, w, -1.0, False)]
        else:
            cb = c - mlo
            segs = [(0, cb, 1.0, True), (cb, w, -1.0, False)]
        out = []
        for lo, hi, sign, is_p in segs:
            if is_p:
                out.append((lo, hi, sign, True, p_cols))
                p_cols += 1
            else:
                out.append((lo, hi, sign, False, n_cols))
                n_cols += 1
        plan.append(out)
    return plan, p_cols, n_cols


def _build_nc(c):
    plan, n_p, n_n = _act_plan(c)

    nc = bacc.Bacc(
        "TRN2",
        target_bir_lowering=False,
        debug=False,
        num_devices=N_CORES,
    )
    xt = nc.dram_tensor("xt", [128, NSUB * 1024], BF16, kind="ExternalInput").ap()
    # free layout: [pair, mb&1, dc, 512] — SBUF-identical, mb-major within
    # a pair so a 512-phase's W is one contiguous region.
    w0t = nc.dram_tensor("w0t", [128, NMB * ND * 512], BF16, kind="ExternalInput").ap()
    abp = nc.dram_tensor("abp", [128, D + M], BF16, kind="ExternalInput").ap()
    b0r = nc.dram_tensor("b0r", [128, M], BF16, kind="ExternalInput").ap()
    b2s = nc.dram_tensor("b2s", [128, 1], F32, kind="ExternalInput").ap()
    # padded: rows 16..31 receive the vector-transpose garbage lanes
    out = nc.dram_tensor("out", [2 * NSUB, 128], F32, kind="ExternalOutput").ap()

    RELU = mybir.ActivationFunctionType.Relu
    ADD = mybir.AluOpType.add
    SUB = mybir.AluOpType.subtract
    MULT = mybir.AluOpType.mult
    AXX = mybir.AxisListType.X

    with tile.TileContext(nc) as tc:
        with (
            tc.tile_pool(name="cp", bufs=1) as cp,
            tc.tile_pool(name="hb", bufs=3) as hb,
            tc.tile_pool(name="rb", bufs=2) as rb,
            tc.tile_pool(name="psh", bufs=2, space="PSUM") as psh,
            tc.tile_pool(name="psl", bufs=1, space="PSUM") as psl,
        ):
            AB = cp.tile([128, D + M], BF16, tag="ab")
            BT = AB[:, 128 : 128 + M]

            def a2s(dc):
                if dc == 0:
                    return AB[:, 0:128]
                return AB[:, 128 + M + (dc - 1) * 128 : 128 + M + dc * 128]
            B0R = cp.tile([128, M], BF16, tag="b0r")
            W = cp.tile([128, NMB * ND * 512], BF16, tag="w")
            X = cp.tile([128, NSUB * 1024], BF16, tag="x")
            B2C = cp.tile([128, 1], F32, tag="b2")
            CORR = cp.tile([128, 8192], BF16, tag="corr")
            # +1 spare column: the last tile splits its boundary-crossing
            # activation segment in two, and accum_out overwrites.
            RSP = cp.tile([128, NSUB * max(n_p, 1) + 1], F32, tag="rsp")
            RSN = cp.tile([128, NSUB * max(n_n, 1) + 1], F32, tag="rsn")
            OUT = cp.tile([128, 2 * NSUB], F32, tag="out")
            OUTT = cp.tile([128, 2 * NSUB], F32, tag="outt")

            def _chunked(dma_fn, lo, hi, step=2048):
                while lo < hi:
                    mid = min(lo + step, hi)
                    dma_fn(lo, mid)
                    lo = mid

            def x_dma(blo, bhi):
                _chunked(
                    lambda lo, hi: nc.sync.dma_start(
                        out=X[:, lo:hi], in_=xt[:, lo:hi]
                    ),
                    blo * 1024,
                    bhi * 1024,
                )

            def w_dma(lo, hi):
                _chunked(
                    lambda lo_, hi_: nc.sync.dma_start(
                        out=W[:, lo_:hi_], in_=w0t[:, lo_:hi_]
                    ),
                    lo,
                    hi,
                )

            def w_off(mb, dc):
                return (mb // 2) * 8192 + (mb % 2) * 4096 + dc * 512

            def lora_unit(mp_, dc):
                """LoRA correction blocks (mb0, dc), (mb1, dc) of pair mp_:
                two K=16 matmuls packed into disjoint PE row bands, one
                single-bank PSUM tile per block (4 tiles in flight)."""
                lps = []
                for i, mb in enumerate((2 * mp_, 2 * mp_ + 1)):
                    # alternate row-band sets per dc parity so consecutive
                    # units hit distinct PE row groups and overlap fully
                    r0 = 64 * (dc % 2) + 32 * i
                    lp = psl.tile([128, 512], F32, tag=f"l{2 * (dc % 2) + i}")
                    nc.tensor.matmul(
                        lp[:],
                        a2s(dc)[r0 : r0 + R, :],
                        BT[r0 : r0 + R, mb * 512 : (mb + 1) * 512],
                        start=True,
                        stop=True,
                        tile_position=(r0, 0),
                    )
                    lps.append(lp)
                return lps

            def lora_group(mp_, dc):
                """In-phase lora: unit matmuls + direct adds into W."""
                lpa, lpb = lora_unit(mp_, dc)
                for mb, lp in ((2 * mp_, lpa), (2 * mp_ + 1, lpb)):
                    off = w_off(mb, dc)
                    nc.vector.tensor_add(
                        W[:, off : off + 512], W[:, off : off + 512], lp[:]
                    )

            # --- prologue DMAs, strict need-order per queue: completions
            # are FIFO, so anything issued earlier delays the semaphore of
            # everything after it.  Sync carries only the tile-gating
            # prefix; scalar/gpsimd queues issue the rest in parallel.
            nc.sync.dma_start(out=AB[:, 0:1152], in_=abp[:, 0:1152])
            nc.sync.dma_start(
                out=AB[:, 128 + M : 128 + M + 896],
                in_=abp[:, 128 + M : 128 + M + 896],
            )
            w_dma(0, 8192)  # pair 0
            x_dma(0, 1)
            x_dma(1, 4)
            nc.scalar.dma_start(out=B0R[:, 0:1024], in_=b0r[:, 0:1024])
            nc.gpsimd.dma_start(out=B2C[:], in_=b2s)
            nc.vector.memset(OUT[:, NSUB : 2 * NSUB], 0.0)

            # PE p-state warmup: dummy matmuls on memset data, runnable
            # from boot (no DMA dependency) so the tensor engine is at
            # full clock when the first real work arrives.  Short fillers
            # are also sprinkled through the prologue (see dummy_mm) —
            # idle gaps reset the clock ramp.
            DUM = cp.tile([128, 640], BF16, tag="dum")
            nc.vector.memset(DUM[:], 0.0)
            _dumk = [0]

            def dummy_mm(cols=512, n=1):
                # fillers cycle the (idle-until-tiles) psh pool so they
                # never contend with the lora psl tiles
                for _ in range(n):
                    dps = psh.tile([128, cols], F32, tag="hp")
                    _dumk[0] += 1
                    nc.tensor.matmul(
                        dps[:],
                        DUM[:, 0:128],
                        DUM[:, 128 : 128 + cols],
                        start=True,
                        stop=True,
                    )

            dummy_mm(cols=512, n=10)

            # LoRA for pair 0: stage corrections in SBUF via Scalar/Vector
            # copies (no W dependency), then cheap bf16 SBUF adds per W
            # chunk as its DMA lands — Vector never head-of-line blocks.
            def corr_add(q):
                off = q * 2048
                nc.vector.tensor_add(
                    W[:, off : off + 2048],
                    W[:, off : off + 2048],
                    CORR[:, off : off + 2048],
                )

            for dc in range(ND):
                lpa, lpb = lora_unit(0, dc)
                nc.scalar.copy(CORR[:, dc * 512 : (dc + 1) * 512], lpa[:])
                nc.vector.tensor_copy(
                    CORR[:, 4096 + dc * 512 : 4096 + (dc + 1) * 512], lpb[:]
                )
                if dc == 3:
                    # dc0-3 corrections complete: unblock the first tiles'
                    # early matmuls (chunks 0 and 2) ahead of the rest
                    corr_add(0)
                    corr_add(2)
            corr_add(1)
            corr_add(3)

            def combine(lo, hi, p_extra=0, n_extra=0):
                """out[b] = sum_P - sum_N + b2 for bsubs [lo, hi) via
                segmented (3D-AP) reduces + one fused scalar op."""
                w = hi - lo
                oc = OUT[:, lo:hi]
                rp = rn = None
                if n_p:
                    rp = rb.tile([128, w], F32, tag="rp")
                    src = RSP[:, lo * n_p : hi * n_p + p_extra]
                    if w > 1:
                        src = src.rearrange("p (b c) -> p b c", c=n_p)
                    nc.vector.tensor_reduce(rp[:], src, AXX, ADD)
                if n_n:
                    rn = rb.tile([128, w], F32, tag="rn")
                    src = RSN[:, lo * n_n : hi * n_n + n_extra]
                    if w > 1:
                        src = src.rearrange("p (b c) -> p b c", c=n_n)
                    nc.vector.tensor_reduce(rn[:], src, AXX, ADD)
                if n_p and n_n:
                    nc.vector.scalar_tensor_tensor(
                        oc, rp[:], B2C[:, 0:1], rn[:], ADD, SUB
                    )
                elif n_p:
                    nc.vector.tensor_scalar_add(oc, rp[:], B2C[:, 0:1])
                else:
                    nc.vector.scalar_tensor_tensor(
                        oc, rn[:], -1.0, B2C[:, 0:1], MULT, ADD
                    )

            for ph, (mlo, mhi) in enumerate(PHASES):
                wph = mhi - mlo
                mbs = [mlo // 512 + k for k in range(wph // 512)]
                last_ph = ph == len(PHASES) - 1
                for bsub in range(NSUB):
                    # paced background DMA / next-phase prep, need-ordered
                    if ph == 0:
                        if bsub == 0:
                            x_dma(4, 10)
                        elif bsub == 4:
                            x_dma(10, NSUB)
                        elif bsub == 8:
                            _chunked(
                                lambda lo, hi: nc.sync.dma_start(
                                    out=AB[:, 128 + lo : 128 + hi],
                                    in_=abp[:, 128 + lo : 128 + hi],
                                ),
                                1024,
                                4096,
                            )
                    if ph <= 2:
                        if bsub == 0:
                            w_dma((ph + 1) * 8192, (ph + 2) * 8192)
                            nc.sync.dma_start(
                                out=B0R[:, (ph + 1) * 1024 : (ph + 2) * 1024],
                                in_=b0r[:, (ph + 1) * 1024 : (ph + 2) * 1024],
                            )
                        if bsub in (8, 10, 12, 14):
                            # adjacent dc pair: their 4 matmuls pack into
                            # the four PE row bands in one span
                            k = bsub - 8
                            lora_group(ph + 1, k)
                            lora_group(ph + 1, k + 1)

                    def mm(hp, si, dc, slot=None):
                        mb = mbs[si]
                        slot = si if slot is None else slot
                        lhsT = X[
                            :, bsub * 1024 + dc * 128 : bsub * 1024 + (dc + 1) * 128
                        ]
                        nc.tensor.matmul(
                            hp[:, slot * 512 : slot * 512 + 512],
                            lhsT,
                            W[:, w_off(mb, dc) : w_off(mb, dc) + 512],
                            start=(dc == 0),
                            stop=(dc == ND - 1),
                        )

                    def bias(hp, lo, hi):
                        nc.vector.tensor_add(
                            hp[:, lo:hi],
                            hp[:, lo:hi],
                            B0R[:, mlo + lo : mlo + hi],
                        )

                    def act(hp, lo, hi, sign, is_p, col, spare=False):
                        hs = hb.tile([128, 1024], BF16, tag="hs")
                        rs = RSP if is_p else RSN
                        ncols = n_p if is_p else n_n
                        c0 = NSUB * ncols if spare else bsub * ncols + col
                        nc.scalar.activation(
                            hs[:, 0 : hi - lo],
                            hp[:, lo:hi],
                            RELU,
                            scale=sign,
                            accum_out=rs[:, c0 : c0 + 1],
                        )

                    if last_ph and bsub == NSUB - 1:
                        # last tile: half-split into two separate PSUM
                        # tiles (tile-granular dep tracking would stall
                        # half-B's matmuls behind half-A's reads); the
                        # boundary-crossing activation segment goes to the
                        # spare accum column.
                        hpa = psh.tile([128, 1024], F32, tag="hp")
                        hpb = psl.tile([128, 512], F32, tag="l0")
                        for dc in range(ND):
                            mm(hpa, 0, dc)
                        for dc in range(ND):
                            mm(hpb, 1, dc, slot=0)
                        bias(hpa, 0, 512)
                        for lo, hi, sign, is_p, col in plan[ph]:
                            if hi <= 512:
                                act(hpa, lo, hi, sign, is_p, col)
                            elif lo < 512:
                                act(hpa, lo, 512, sign, is_p, col)
                        combine(0, NSUB - 1)
                        nc.vector.tensor_add(
                            hpb[:, 0:512],
                            hpb[:, 0:512],
                            B0R[:, mlo + 512 : mlo + 1024],
                        )
                        p_extra = n_extra = 0
                        for lo, hi, sign, is_p, col in plan[ph]:
                            if lo >= 512:
                                act(hpb, lo - 512, hi - 512, sign, is_p, col)
                            elif hi > 512:
                                act(hpb, 0, hi - 512, sign, is_p, col, spare=True)
                                if is_p:
                                    p_extra = 1
                                else:
                                    n_extra = 1
                        combine(NSUB - 1, NSUB, p_extra, n_extra)
                    else:
                        hp = psh.tile([128, wph], F32, tag="hp")
                        for dc in range(ND):
                            for si in range(len(mbs)):
                                mm(hp, si, dc)
                        bias(hp, 0, wph)
                        for lo, hi, sign, is_p, col in plan[ph]:
                            act(hp, lo, hi, sign, is_p, col)

            # output: 32x32 block transpose on VectorE, then one DMA whose
            # DRAM AP de-interleaves the blocks; OUTT[32i+c, r] holds
            # y[bsub=c, b=32i+r], so dram row c gets (i r).
            nc.vector.transpose(OUTT[:], OUT[:])
            nc.sync.dma_start(
                out=out.rearrange("c (i r) -> i c r", i=4),
                in_=OUTT[:, 0 : 2 * NSUB],
            )

    nc.compile()
    return nc


def _prep_in_maps(x, W0, b0, A, B, W2, b2, c, perm):
    w2 = W2[0]
    W0p = (W0 * w2[:, None])[perm]
    Bp = (B * w2[:, None])[perm]
    b0p = (b0 * w2)[perm]

    # [128, mp*8192 + s*4096 + dc*512 + j] = W0p[(2mp+s)*512 + j, dc*128 + p]
    w0t = np.ascontiguousarray(
        W0p.reshape(NMP, 2, 512, ND, 128)
        .transpose(4, 0, 1, 3, 2)
        .reshape(128, NMB * ND * 512)
    ).astype(NP_BF16)
    # lora operands replicated into the four 32-row PE tile bands,
    # concatenated [A2 | BT] so one DMA covers the first unit's needs
    abp = np.zeros((128, D + M), dtype=NP_BF16)
    a2full = (SCALING * A).astype(NP_BF16)
    btfull = Bp.T.astype(NP_BF16)
    for i in range(4):
        abp[32 * i : 32 * i + R, 0:128] = a2full[:, 0:128]
        abp[32 * i : 32 * i + R, 128 : 128 + M] = btfull
        abp[32 * i : 32 * i + R, 128 + M :] = a2full[:, 128:]
    b0rep = np.ascontiguousarray(
        np.broadcast_to(b0p.astype(NP_BF16)[None, :], (128, M))
    )
    b2s = np.full((128, 1), b2[0], dtype=np.float32)

    in_maps = []
    for cix in range(N_CORES):
        xs = x[cix * BS : (cix + 1) * BS]  # [2048, 1024]
        # xt[p, bsub*1024 + dc*128 + bb] = xs[bsub*128 + bb, dc*128 + p]
        xt = np.ascontiguousarray(
            xs.reshape(NSUB, 128, ND, 128).transpose(3, 0, 2, 1).reshape(128, NSUB * 1024)
        ).astype(NP_BF16)
        in_maps.append(
            {
                "xt": xt,
                "w0t": w0t,
                "abp": abp,
                "b0r": b0rep,
                "b2s": b2s,
            }
        )
    return in_maps


def kernel(x, W0, b0, A, B, W2, b2, _trace=False, _trace_kwargs=None):
    x = np.asarray(x, dtype=np.float32)
    W0 = np.asarray(W0, dtype=np.float32)
    b0 = np.asarray(b0, dtype=np.float32)
    A = np.asarray(A, dtype=np.float32)
    B = np.asarray(B, dtype=np.float32)
    W2 = np.asarray(W2, dtype=np.float32)
    b2 = np.asarray(b2, dtype=np.float32)

    w2 = W2[0]
    pos = w2 >= 0
    c = int(pos.sum())
    perm = np.concatenate([np.where(pos)[0], np.where(~pos)[0]])

    key = ("nc", c)
    if key not in _CACHE:
        _CACHE[key] = _build_nc(c)
    nc = _CACHE[key]

    in_maps = _prep_in_maps(x, W0, b0, A, B, W2, b2, c, perm)
    res = run_bass_kernel_spmd(
        nc,
        in_maps,
        list(range(N_CORES)),
        trace=_trace,
        **(_trace_kwargs or {}),
    )
    out = np.concatenate([r["out"][:NSUB].reshape(BS) for r in res.results])
    if _trace:
        _CACHE["last_results"] = res
    return out.astype(np.float32)
